# revision 1
# baseline (speedup 1.0000x reference)
"""Trainium2 Bass kernel for attention-weighted pooling.

Computes, for x[B,T,D], W[D,1], b[T,1]:
    et = tanh(x @ W + b)            # (B, T)
    at = softmax(et, axis=-1)       # (B, T)
    out = einsum('btd,bt->bd', x, at)

Sharding: pure data parallel over batch across 8 NeuronCores (4 batches per
core); W and b replicated. No collectives.

Key structure (per core, streaming single pass over x):
  - tanh output is bounded in [-1, 1], so softmax needs no max subtraction;
    exp() cannot overflow. Normalization by the denominator is deferred to
    the very end, so x is read from HBM exactly once (memory roofline).
  - x is cast fp32 -> bf16 during the DMA itself (SWDGE cast): HBM traffic
    stays the required 32 MiB/core of fp32, but on-chip x is half the bytes.
    This makes the PE matmul single-pass (fp32 matmul lowers to an HI/LO
    pass PAIR and measured ~135us of PE time - always above the ~93us DMA
    roofline) and makes the DVE dot-product eligible for the 2x bf16 mode.
  - Per 1-MiB(HBM) super-tile [128 x (4*512)]: fused DVE
    scalar_tensor_tensor gives elin[t] = sum_d x[t,d]*W[d] and the x*W
    products in one pass; DVE adds b; ACT does tanh then exp (p in bf16);
    PE accumulates p.T @ x_tile into PSUM [1, D].
  - Per-batch epilogue: S = sum_t p_t (ones-matmul), out = acc / S.
"""

import sys

sys.path.insert(0, "/opt/trn_rl_repo")

import numpy as np

B, T, D = 32, 4096, 512
N_CORES = 8
B_LOCAL = B // N_CORES          # 4 batches per core
P = 128                         # SBUF partitions
TS_T = 1024                     # t-rows per super-tile (2 MiB fp32 DMA)
N_ST = T // TS_T                # 4 super-tiles per batch
N_J = TS_T // P                 # 8 t-subtiles per super-tile
N_STT = 4                       # subtiles 0..N_STT-1 use the fused DVE op;
                                # the rest use DVE mult + ACT accum-reduce

_PROGRAM = None


def _build_program():
    import concourse.bacc as bacc
    import concourse.mybir as mybir
    import concourse.tile as tile

    f32 = mybir.dt.float32
    bf16 = mybir.dt.float16
    nc = bacc.Bacc("TRN2", target_bir_lowering=False, debug=False)

    x_d = nc.dram_tensor("x", [B_LOCAL, T, D], f32, kind="ExternalInput")
    W_d = nc.dram_tensor("W", [D, 1], f32, kind="ExternalInput")
    b_d = nc.dram_tensor("b", [T, 1], f32, kind="ExternalInput")
    o_d = nc.dram_tensor("out", [B_LOCAL, D], f32, kind="ExternalOutput")

    with tile.TileContext(nc) as tc:
        with (
            tc.tile_pool(name="consts", bufs=1) as consts,
            tc.tile_pool(name="xin", bufs=6) as xin,
            tc.tile_pool(name="scratch", bufs=2) as scratch_pool,
            tc.tile_pool(name="prod", bufs=2) as prod_pool,
            tc.tile_pool(name="small", bufs=2) as small,
            tc.tile_pool(name="pbuf", bufs=2) as pbuf_pool,
            tc.tile_pool(name="acc_psum", bufs=2, space="PSUM") as acc_psum_pool,
            tc.tile_pool(name="s_psum", bufs=2, space="PSUM") as s_psum_pool,
        ):
            # Per-batch tile plan: (t0, n_j) chunks. Full 2-MiB tiles except
            # the tail of the LAST batch, which tapers so the final
            # DVE->ACT->PE chain after the last DMA is short.
            full = [(t0, N_J) for t0 in range(0, T, TS_T)]
            plans = [full] * B_LOCAL

            # W broadcast to all 128 partitions, cast to bf16: [128, D]
            w_bcast = consts.tile([P, D], bf16)
            nc.gpsimd.dma_start(
                w_bcast[:],
                W_d.ap().rearrange("d one -> one d").broadcast_to([P, D]),
            )
            # b laid out to match t = st*TS_T + j*P + p: [128, N_ST*N_J, 1]
            b_buf = consts.tile([P, N_ST * N_J, 1], f32)
            nc.sync.dma_start(
                b_buf[:],
                b_d.ap().rearrange("(st j p) one -> p (st j) one", st=N_ST, j=N_J, p=P),
            )
            ones_col = consts.tile([P, 1], f32)
            nc.vector.memset(ones_col[:], 1.0)

            for bb in range(B_LOCAL):
                p_buf = pbuf_pool.tile([P, T // P], bf16)
                acc = acc_psum_pool.tile([1, D], f32)

                chunks = plans[bb]
                total_mm = sum(nj for _, nj in chunks)
                mm_idx = 0
                for ci, (t0, nj) in enumerate(chunks):
                    col0 = t0 // P
                    # SWDGE dma with inline fp32->fp16 cast
                    xt = xin.tile([P, nj, D], bf16, tag="xt")
                    nc.gpsimd.dma_start(
                        xt[:],
                        x_d.ap()[bb, t0 : t0 + nj * P, :].rearrange(
                            "(j p) d -> p j d", p=P
                        ),
                    )
                    elin = small.tile([P, nj], f32)
                    # Half the subtiles: fused mult+reduce on DVE
                    # (scalar_tensor_tensor, 1x uop ~690ns). Other half: plain
                    # tensor_tensor mult on DVE (fp16 2x_1P mode, ~360ns) with
                    # the reduce offloaded to ACT (activation Copy +
                    # accum_out, ~1.0us incl accumulator read). This splits
                    # the per-element dot-product work so both engines stay
                    # under the DMA roofline.
                    n_stt = nj // 2
                    for j in range(n_stt):
                        scratch = scratch_pool.tile([P, D], bf16)
                        nc.vector.scalar_tensor_tensor(
                            out=scratch[:],
                            in0=xt[:, j, :],
                            scalar=1.0,
                            in1=w_bcast[:],
                            op0=mybir.AluOpType.mult,
                            op1=mybir.AluOpType.mult,
                            accum_out=elin[:, j : j + 1],
                        )
                    for j in range(n_stt, nj):
                        prod = prod_pool.tile([P, D], bf16)
                        nc.vector.tensor_mul(prod[:], xt[:, j, :], w_bcast[:])
                        nc.scalar.activation(
                            prod[:],
                            prod[:],
                            mybir.ActivationFunctionType.Copy,
                            accum_out=elin[:, j : j + 1],
                        )
                    for ws, wn in ((0, nj),):
                        ee = small.tile([P, wn], f32)
                        nc.vector.tensor_add(
                            ee[:],
                            elin[:, ws : ws + wn],
                            b_buf[:, col0 + ws : col0 + ws + wn, 0],
                        )
                        et = small.tile([P, wn], f32)
                        nc.scalar.activation(
                            et[:], ee[:], mybir.ActivationFunctionType.Tanh
                        )
                        nc.scalar.activation(
                            p_buf[:, col0 + ws : col0 + ws + wn],
                            et[:],
                            mybir.ActivationFunctionType.Exp,
                        )
                        for j in range(ws, ws + wn):
                            nc.tensor.matmul(
                                acc[:],
                                p_buf[:, col0 + j : col0 + j + 1],
                                xt[:, j, :],
                                start=(mm_idx == 0),
                                stop=(mm_idx == total_mm - 1),
                            )
                            mm_idx += 1

                # denominator S = sum_t p_t  (free-dim reduce, then
                # cross-partition reduce via ones-matmul)
                ssum = small.tile([P, 1], f32)
                nc.vector.reduce_sum(ssum[:], p_buf[:], axis=mybir.AxisListType.X)
                s_ps = s_psum_pool.tile([1, 1], f32)
                nc.tensor.matmul(s_ps[:], ssum[:], ones_col[:])
                sinv = small.tile([1, 1], f32)
                nc.vector.reciprocal(sinv[:], s_ps[:])
                out_sb = small.tile([1, D], f32)
                nc.scalar.mul(out_sb[:], acc[:], sinv[:])
                nc.sync.dma_start(o_d.ap()[bb : bb + 1, :], out_sb[:])

    nc.compile()
    return nc


def _get_program():
    global _PROGRAM
    if _PROGRAM is None:
        _PROGRAM = _build_program()
    return _PROGRAM


def _shard_inputs(x, W, b):
    x = np.ascontiguousarray(np.asarray(x, dtype=np.float32))
    W = np.ascontiguousarray(np.asarray(W, dtype=np.float32))
    b = np.ascontiguousarray(np.asarray(b, dtype=np.float32))
    return [
        {"x": x[c * B_LOCAL : (c + 1) * B_LOCAL], "W": W, "b": b}
        for c in range(N_CORES)
    ]


def _install_ntff_hook_shim():
    """The agent image's ``antenv`` lacks ``axon_hooks``, so the boot-time
    NTFF hook registration silently degrades. Recreate the module in
    sys.modules and register the ctypes hook against libaxon_pjrt.so."""
    import types

    if "antenv.axon_hooks" in sys.modules:
        return
    mod = types.ModuleType("antenv.axon_hooks")
    _hook = [None]
    mod.set_axon_ntff_profile_hook = lambda h: _hook.__setitem__(0, h)
    mod.get_axon_ntff_profile_hook = lambda: _hook[0]
    import antenv

    antenv.axon_hooks = mod
    sys.modules["antenv.axon_hooks"] = mod
    try:
        sys.path.insert(0, "/root/.axon_site")
        from trn_agent_boot.trn_boot import _ntff_profile_via_ctypes

        mod.set_axon_ntff_profile_hook(
            _ntff_profile_via_ctypes("/opt/axon/libaxon_pjrt.so")
        )
    except Exception as e:  # profiling is best-effort; run still works
        print(f"NTFF hook shim failed ({e}); tracing disabled", file=sys.stderr)


def _run(in_maps, trace=False):
    from concourse.bass_utils import run_bass_kernel_spmd

    nc = _get_program()
    kwargs = {}
    if trace:
        _install_ntff_hook_shim()
        kwargs = {"trace": True, "trace_cores": [0]}
    return run_bass_kernel_spmd(nc, in_maps, core_ids=list(range(N_CORES)), **kwargs)


def kernel(x, W, b):
    res = _run(_shard_inputs(x, W, b))
    return np.concatenate(
        [res.results[c]["out"] for c in range(N_CORES)], axis=0
    ).astype(np.float32)


def kernel_profiled(x, W, b):
    """Like kernel() but also returns the NTFF-measured exec time in ns."""
    res = _run(_shard_inputs(x, W, b), trace=True)
    out = np.concatenate(
        [res.results[c]["out"] for c in range(N_CORES)], axis=0
    ).astype(np.float32)
    return out, res

